# revision 6
# baseline (speedup 1.0000x reference)
"""Masked causal attention (B=2, T=2048, C=1024, N=16 heads, D=64) on 8 TRN2 cores.

Sharding: tensor-parallel over heads. Core c computes heads 2c, 2c+1 (a
contiguous 128-channel block) for both batches: Q/K/V projections for its
head block, causal-masked softmax attention, and its partial contribution
to the output projection (Wo rows for its channels). The host sums the 8
partial outputs and adds bo.

Per-core dataflow (all matmuls in fp32r; contraction always on partitions):
  srcT [C, M]  -> Q^T, K^T, V^T [128, M] via lhsT=W-slices, rhs=srcT chunks
  V^T -> V [s, 128] via PE identity transpose (free dim stays >= 256 for
  every big matmul, which is what keeps fp32r at 1 cycle/row)
  scores^T [s, t] = (K^T slice).T @ Q^T chunk   (contraction over d=64)
  exp via ACT (scale 1/sqrt(D) folded into the Q copyback), causal mask
  applied as an additive -3e4 strip on the diagonal blocks
  attn_out^T and the softmax denominator come from one matmul per s-tile:
  lhsT = [v_head | ones] (65 cols) -> psum rows 0..63 = unnorm out^T,
  row 64 = sum of exp.  Normalize with reciprocal + partition_broadcast.
  o-proj: psum[t,c] = aoT.T @ Wo_slice, written to DRAM as fp32 partial.
"""

import sys

sys.path.insert(0, "/opt/trn_rl_repo")

import numpy as np

B, T, C = 2, 2048, 1024
NHEADS = 16
D = 64
M = B * T          # 4096 flattened rows
P = 128            # partitions
KC = C // P        # 8 contraction tiles
TC = 512           # t-chunk (matmul free dim)
NMC = M // TC      # 8 m-chunks
NST = T // P       # 16 s-tiles per batch
NTC = T // TC      # 4 t-chunks per batch
MASK_NEG = -30000.0

_CACHE = {}


def _build_program():
    import concourse.bass as bass
    from concourse import bacc
    import concourse.mybir as mybir
    from concourse.tile import TileContext

    dt = mybir.dt
    nc = bacc.Bacc("TRN2", target_bir_lowering=False, debug=False, num_devices=8)

    srct = nc.dram_tensor("srct", [C, M], dt.float32r, kind="ExternalInput")
    wqkv = nc.dram_tensor("wqkv", [C, 3 * P], dt.float32r, kind="ExternalInput")
    wo = nc.dram_tensor("wo", [P, C], dt.float32r, kind="ExternalInput")
    bias = nc.dram_tensor("bias", [P, 3], dt.float32, kind="ExternalInput")
    m0 = nc.dram_tensor("m0", [P, 1024], dt.float32, kind="ExternalInput")
    ident = nc.dram_tensor("ident", [P, P], dt.float32r, kind="ExternalInput")
    ones = nc.dram_tensor("ones", [P, B * NST], dt.float32r, kind="ExternalInput")
    out = nc.dram_tensor("out", [M, C], dt.float32, kind="ExternalOutput")

    srct_t = srct.ap().rearrange("(ko p) m -> p ko m", p=P)
    wqkv_t = wqkv.ap().rearrange("(ko p) j -> p ko j", p=P)

    ACT_COPY = mybir.ActivationFunctionType.Identity
    ACT_EXP = mybir.ActivationFunctionType.Exp

    with TileContext(nc) as tc:
        with (
            tc.tile_pool(name="persist", bufs=1) as persist,
            tc.tile_pool(name="srcp", bufs=2) as srcp,
            tc.tile_pool(name="vtp", bufs=2) as vtp,
            tc.tile_pool(name="ep", bufs=4) as ep,
            tc.tile_pool(name="nrm", bufs=2) as nrm,
            tc.tile_pool(name="outp", bufs=3) as outp,
            tc.tile_pool(name="ps", bufs=7, space="PSUM") as ps,
        ):
            # ---- persistent SBUF tensors ----
            wqkv_sb = persist.tile([P, KC, 3 * P], dt.float32r, name="wqkv_sb")
            nc.sync.dma_start(out=wqkv_sb[:], in_=wqkv_t)
            wo_sb = persist.tile([P, C], dt.float32r, name="wo_sb")
            nc.sync.dma_start(out=wo_sb[:], in_=wo.ap())
            bias_sb = persist.tile([P, 3], dt.float32, name="bias_sb")
            nc.sync.dma_start(out=bias_sb[:], in_=bias.ap())
            m0_sb = persist.tile([P, 1024], dt.float32, name="m0_sb")
            nc.sync.dma_start(out=m0_sb[:], in_=m0.ap())
            ident_sb = persist.tile([P, P], dt.float32r, name="ident_sb")
            nc.sync.dma_start(out=ident_sb[:], in_=ident.ap())

            qT_sb = persist.tile([P, M], dt.float32r, name="qT_sb")
            kT_sb = persist.tile([P, M], dt.float32r, name="kT_sb")
            # v layout per s-tile: [vA(0:64) | ones(64) | vB(65:129) | ones(129)]
            v_sb = persist.tile([P, B * NST, 130], dt.float32r, name="v_sb")
            nc.sync.dma_start(out=v_sb[:, :, 64], in_=ones.ap())
            nc.sync.dma_start(out=v_sb[:, :, 129], in_=ones.ap())
            aoT_sb = persist.tile([P, M], dt.float32r, name="aoT_sb")

            # ---- phase B: projections (+ V transpose) ----
            for mc in range(NMC):
                msl = slice(mc * TC, (mc + 1) * TC)
                src_sb = srcp.tile([P, KC, TC], dt.float32r, name="src_sb")
                nc.sync.dma_start(out=src_sb[:], in_=srct_t[:, :, msl])

                ps_q = ps.tile([P, TC], dt.float32, name="ps_q", tag="ps")
                for ko in range(KC):
                    nc.tensor.matmul(
                        ps_q[:], wqkv_sb[:, ko, 0:P], src_sb[:, ko, :],
                        start=(ko == 0), stop=(ko == KC - 1),
                    )
                nc.scalar.activation(
                    qT_sb[:, msl], ps_q[:], ACT_COPY,
                    bias=bias_sb[:, 0:1], scale=0.125,
                )

                ps_k = ps.tile([P, TC], dt.float32, name="ps_k", tag="ps")
                for ko in range(KC):
                    nc.tensor.matmul(
                        ps_k[:], wqkv_sb[:, ko, P:2 * P], src_sb[:, ko, :],
                        start=(ko == 0), stop=(ko == KC - 1),
                    )
                nc.scalar.activation(
                    kT_sb[:, msl], ps_k[:], ACT_COPY, bias=bias_sb[:, 1:2],
                )

                ps_v = ps.tile([P, TC], dt.float32, name="ps_v", tag="ps")
                for ko in range(KC):
                    nc.tensor.matmul(
                        ps_v[:], wqkv_sb[:, ko, 2 * P:3 * P], src_sb[:, ko, :],
                        start=(ko == 0), stop=(ko == KC - 1),
                    )
                vt_sb = vtp.tile([P, TC], dt.float32r, name="vt_sb")
                nc.scalar.activation(
                    vt_sb[:], ps_v[:], ACT_COPY, bias=bias_sb[:, 2:3],
                )
                # transpose the 4 [128,128] blocks of this chunk into v_sb
                for k4 in range(TC // P):
                    st = mc * (TC // P) + k4
                    ps_t = ps.tile([P, P], dt.float32r, name="ps_t", tag="ps")
                    nc.tensor.transpose(
                        ps_t[:], vt_sb[:, k4 * P:(k4 + 1) * P], ident_sb[:]
                    )
                    nc.vector.tensor_copy(v_sb[:, st, 0:64], ps_t[:, 0:64])
                    nc.vector.tensor_copy(v_sb[:, st, 65:129], ps_t[:, 64:128])

            # ---- phase C/D: attention + output projection, per batch ----
            for b in range(B):
                for tci in range(NTC):
                    t0 = b * T + tci * TC
                    tsl = slice(t0, t0 + TC)
                    n_st = (tci + 1) * (TC // P)
                    for h in range(2):
                        jh = h * 64
                        ps_o = ps.tile([P, TC], dt.float32, name="ps_o", tag="ps")
                        for st in range(n_st):
                            s0 = st * P
                            ps_s = ps.tile([P, TC], dt.float32, name="ps_s", tag="ps")
                            nc.tensor.matmul(
                                ps_s[:],
                                kT_sb[jh:jh + 64, b * T + s0:b * T + s0 + P],
                                qT_sb[jh:jh + 64, tsl],
                                start=True, stop=True,
                            )
                            dd = s0 - tci * TC
                            if dd >= 0:  # diagonal block: additive causal mask
                                o = 384 - dd
                                nc.vector.tensor_tensor(
                                    ps_s[:], ps_s[:], m0_sb[:, o:o + TC],
                                    mybir.AluOpType.add,
                                )
                            e_sb = ep.tile([P, TC], dt.float32r, name="e_sb")
                            nc.scalar.activation(e_sb[:], ps_s[:], ACT_EXP)
                            vcol = h * 65
                            nc.tensor.matmul(
                                ps_o[0:65, :],
                                v_sb[:, b * NST + st, vcol:vcol + 65],
                                e_sb[:],
                                start=(st == 0), stop=(st == n_st - 1),
                            )
                        # normalize: rows 0..63 / row 64
                        rc_sb = nrm.tile([65, TC], dt.float32, name="rc_sb")
                        nc.vector.reciprocal(rc_sb[64:65, :], ps_o[64:65, :])
                        rc0_sb = nrm.tile([1, TC], dt.float32, name="rc0_sb")
                        nc.sync.dma_start(out=rc0_sb[:], in_=rc_sb[64:65, :])
                        rb_sb = nrm.tile([64, TC], dt.float32, name="rb_sb")
                        nc.gpsimd.partition_broadcast(rb_sb[:], rc0_sb[:])
                        nc.vector.tensor_tensor(
                            aoT_sb[jh:jh + 64, tsl], ps_o[0:64, :], rb_sb[:],
                            mybir.AluOpType.mult,
                        )
                # output projection for this batch's rows
                for mt in range(b * (T // P), (b + 1) * (T // P)):
                    for cc in range(C // TC):
                        ps_p = ps.tile([P, TC], dt.float32, name="ps_p", tag="ps")
                        nc.tensor.matmul(
                            ps_p[:],
                            aoT_sb[:, mt * P:(mt + 1) * P],
                            wo_sb[:, cc * TC:(cc + 1) * TC],
                            start=True, stop=True,
                        )
                        o_sb = outp.tile([P, TC], dt.float32, name="o_sb")
                        nc.vector.tensor_copy(o_sb[:], ps_p[:])
                        nc.sync.dma_start(
                            out=out.ap()[mt * P:(mt + 1) * P, cc * TC:(cc + 1) * TC],
                            in_=o_sb[:],
                        )

    nc.compile()
    return nc


def _host_inputs(src, mask, Wq, bq, Wk, bk, Wv, bv, Wo, bo):
    f32 = np.float32
    src = np.asarray(src, f32)
    srct = np.ascontiguousarray(src.reshape(M, C).T)

    # causal strip: m0[s, u] = 0 (keep) iff u >= s + 384, else MASK_NEG
    u = np.arange(1024)[None, :]
    s = np.arange(P)[:, None]
    m0 = np.where(u >= s + 384, 0.0, MASK_NEG).astype(f32)
    ident = np.eye(P, dtype=f32)

    in_maps = []
    for c in range(8):
        sl = slice(c * P, (c + 1) * P)
        wqkv = np.concatenate(
            [np.asarray(Wq, f32)[:, sl], np.asarray(Wk, f32)[:, sl],
             np.asarray(Wv, f32)[:, sl]], axis=1,
        )
        bias = np.stack(
            [np.asarray(bq, f32)[sl] * 0.125, np.asarray(bk, f32)[sl],
             np.asarray(bv, f32)[sl]], axis=1,
        ).astype(f32)
        wo_c = np.ascontiguousarray(np.asarray(Wo, f32)[sl, :])
        in_maps.append({
            "srct": srct, "wqkv": np.ascontiguousarray(wqkv), "wo": wo_c,
            "bias": np.ascontiguousarray(bias), "m0": m0, "ident": ident,
            "ones": np.ones((P, B * NST), f32),
        })
    return in_maps


def kernel(src, mask, Wq, bq, Wk, bk, Wv, bv, Wo, bo):
    from concourse.bass_utils import run_bass_kernel_spmd

    if "nc" not in _CACHE:
        _CACHE["nc"] = _build_program()
    nc = _CACHE["nc"]

    in_maps = _host_inputs(src, mask, Wq, bq, Wk, bk, Wv, bv, Wo, bo)
    res = run_bass_kernel_spmd(nc, in_maps, list(range(8)))

    acc = np.zeros((M, C), np.float64)
    for c in range(8):
        acc += res.results[c]["out"]
    acc += np.asarray(bo, np.float64)[None, :]
    return acc.astype(np.float32).reshape(B, T, C)


# revision 11
# speedup vs baseline: 595.9212x; 595.9212x over previous
"""Masked causal attention (B=2, T=2048, C=1024, N=16 heads, D=64) on 8 TRN2 cores.

Sharding: tensor-parallel over heads. Core c computes heads 2c, 2c+1 (a
contiguous 128-channel block) for both batches: Q/K/V projections for its
head block, causal-masked softmax attention, and its partial contribution
to the output projection (Wo rows for its channels). The host sums the 8
partial outputs and adds bo.

Per-core dataflow (all matmuls in fp32r; contraction always on partitions):
  srcT [C, M]  -> Q^T, K^T, V^T [128, M] via lhsT=W-slices, rhs=srcT chunks
  V^T -> V [s, 128] via PE identity transpose (free dim stays >= 256 for
  every big matmul, which is what keeps fp32r at 1 cycle/row)
  scores^T [s, t] = (K^T slice).T @ Q^T chunk   (contraction over d=64)
  exp via ACT (scale 1/sqrt(D) folded into the Q copyback), causal mask
  applied as an additive -3e4 strip on the diagonal blocks
  attn_out^T and the softmax denominator come from one matmul per s-tile:
  lhsT = [v_head | ones] (65 cols) -> psum rows 0..63 = unnorm out^T,
  row 64 = sum of exp.  Normalize with reciprocal + partition_broadcast.
  o-proj: psum[t,c] = aoT.T @ Wo_slice, written to DRAM as fp32 partial.
"""

import sys

sys.path.insert(0, "/opt/trn_rl_repo")

import numpy as np

B, T, C = 2, 2048, 1024
NHEADS = 16
D = 64
M = B * T          # 4096 flattened rows
P = 128            # partitions
KC = C // P        # 8 contraction tiles
TC = 512           # t-chunk (matmul free dim)
NMC = M // TC      # 8 m-chunks
NST = T // P       # 16 s-tiles per batch
NTC = T // TC      # 4 t-chunks per batch
MASK_NEG = -30000.0

_CACHE = {}


def _build_program(repeat=1):
    import concourse.bass as bass
    from concourse import bacc
    import concourse.mybir as mybir
    from concourse.tile import TileContext

    dt = mybir.dt
    nc = bacc.Bacc("TRN2", target_bir_lowering=False, debug=False, num_devices=8)

    srct = nc.dram_tensor("srct", [C, M], dt.float32r, kind="ExternalInput")
    wqkv = nc.dram_tensor("wqkv", [C, 3 * P], dt.float32r, kind="ExternalInput")
    wo = nc.dram_tensor("wo", [P, C], dt.float32r, kind="ExternalInput")
    bias = nc.dram_tensor("bias", [P, 3], dt.float32, kind="ExternalInput")
    m0 = nc.dram_tensor("m0", [P, 1024], dt.float32, kind="ExternalInput")
    ident = nc.dram_tensor("ident", [P, P], dt.float32r, kind="ExternalInput")
    ones = nc.dram_tensor("ones", [P, B * NST], dt.float32r, kind="ExternalInput")
    out = nc.dram_tensor("out", [M, C], dt.float32, kind="ExternalOutput")

    srct_t = srct.ap().rearrange("(ko p) m -> p ko m", p=P)
    wqkv_t = wqkv.ap().rearrange("(ko p) j -> p ko j", p=P)

    ACT_COPY = mybir.ActivationFunctionType.Identity
    ACT_EXP = mybir.ActivationFunctionType.Exp

    with TileContext(nc) as tc:
        with (
            tc.tile_pool(name="persist", bufs=1) as persist,
            tc.tile_pool(name="srcp", bufs=2) as srcp,
            tc.tile_pool(name="vtp", bufs=2) as vtp,
            tc.tile_pool(name="ep", bufs=4) as ep,
            tc.tile_pool(name="nrm", bufs=2) as nrm,
            tc.tile_pool(name="outp", bufs=3) as outp,
            tc.tile_pool(name="ps", bufs=7, space="PSUM") as ps,
        ):
            # ---- persistent SBUF tensors ----
            wqkv_sb = persist.tile([P, KC, 3 * P], dt.float32r, name="wqkv_sb")
            nc.sync.dma_start(out=wqkv_sb[:], in_=wqkv_t)
            wo_sb = persist.tile([P, C], dt.float32r, name="wo_sb")
            nc.sync.dma_start(out=wo_sb[:], in_=wo.ap())
            bias_sb = persist.tile([P, 3], dt.float32, name="bias_sb")
            nc.sync.dma_start(out=bias_sb[:], in_=bias.ap())
            m0_sb = persist.tile([P, 1024], dt.float32, name="m0_sb")
            nc.sync.dma_start(out=m0_sb[:], in_=m0.ap())
            ident_sb = persist.tile([P, P], dt.float32r, name="ident_sb")
            nc.sync.dma_start(out=ident_sb[:], in_=ident.ap())

            qT_sb = persist.tile([P, M], dt.float32r, name="qT_sb")
            kT_sb = persist.tile([P, M], dt.float32r, name="kT_sb")
            # v layout per s-tile: [vA(0:64) | ones(64) | vB(65:129) | ones(129)]
            v_sb = persist.tile([P, B * NST, 130], dt.float32r, name="v_sb")
            nc.sync.dma_start(out=v_sb[:, :, 64], in_=ones.ap())
            nc.sync.dma_start(out=v_sb[:, :, 129], in_=ones.ap())
            aoT_sb = persist.tile([P, M], dt.float32r, name="aoT_sb")

            # ---- phase B: projections (+ V transpose) ----
            def emit_compute():
              for mc in range(NMC):
                msl = slice(mc * TC, (mc + 1) * TC)
                src_sb = srcp.tile([P, KC, TC], dt.float32r, name="src_sb")
                nc.sync.dma_start(out=src_sb[:], in_=srct_t[:, :, msl])

                ps_q = ps.tile([P, TC], dt.float32, name="ps_q", tag="ps")
                for ko in range(KC):
                    nc.tensor.matmul(
                        ps_q[:], wqkv_sb[:, ko, 0:P], src_sb[:, ko, :],
                        start=(ko == 0), stop=(ko == KC - 1),
                    )
                nc.scalar.activation(
                    qT_sb[:, msl], ps_q[:], ACT_COPY,
                    bias=bias_sb[:, 0:1], scale=0.125,
                )

                ps_k = ps.tile([P, TC], dt.float32, name="ps_k", tag="ps")
                for ko in range(KC):
                    nc.tensor.matmul(
                        ps_k[:], wqkv_sb[:, ko, P:2 * P], src_sb[:, ko, :],
                        start=(ko == 0), stop=(ko == KC - 1),
                    )
                nc.scalar.activation(
                    kT_sb[:, msl], ps_k[:], ACT_COPY, bias=bias_sb[:, 1:2],
                )

                ps_v = ps.tile([P, TC], dt.float32, name="ps_v", tag="ps")
                for ko in range(KC):
                    nc.tensor.matmul(
                        ps_v[:], wqkv_sb[:, ko, 2 * P:3 * P], src_sb[:, ko, :],
                        start=(ko == 0), stop=(ko == KC - 1),
                    )
                vt_sb = vtp.tile([P, TC], dt.float32r, name="vt_sb")
                nc.scalar.activation(
                    vt_sb[:], ps_v[:], ACT_COPY, bias=bias_sb[:, 2:3],
                )
                # transpose the 4 [128,128] blocks of this chunk into v_sb
                for k4 in range(TC // P):
                    st = mc * (TC // P) + k4
                    ps_t = ps.tile([P, P], dt.float32r, name="ps_t", tag="ps")
                    nc.tensor.transpose(
                        ps_t[:], vt_sb[:, k4 * P:(k4 + 1) * P], ident_sb[:]
                    )
                    nc.vector.tensor_copy(v_sb[:, st, 0:64], ps_t[:, 0:64])
                    nc.vector.tensor_copy(v_sb[:, st, 65:129], ps_t[:, 64:128])

              # ---- phase C/D: attention + output projection, per batch ----
              for b in range(B):
                for tci in range(NTC):
                    t0 = b * T + tci * TC
                    tsl = slice(t0, t0 + TC)
                    n_st = (tci + 1) * (TC // P)
                    for h in range(2):
                        jh = h * 64
                        ps_o = ps.tile([P, TC], dt.float32, name="ps_o", tag="ps")
                        for st in range(n_st):
                            s0 = st * P
                            ps_s = ps.tile([P, TC], dt.float32, name="ps_s", tag="ps")
                            nc.tensor.matmul(
                                ps_s[:],
                                kT_sb[jh:jh + 64, b * T + s0:b * T + s0 + P],
                                qT_sb[jh:jh + 64, tsl],
                                start=True, stop=True,
                            )
                            dd = s0 - tci * TC
                            if dd >= 0:  # diagonal block: additive causal mask
                                o = 384 - dd
                                nc.vector.tensor_tensor(
                                    ps_s[:], ps_s[:], m0_sb[:, o:o + TC],
                                    mybir.AluOpType.add,
                                )
                            e_sb = ep.tile([P, TC], dt.float32r, name="e_sb")
                            nc.scalar.activation(e_sb[:], ps_s[:], ACT_EXP)
                            vcol = h * 65
                            nc.tensor.matmul(
                                ps_o[0:65, :],
                                v_sb[:, b * NST + st, vcol:vcol + 65],
                                e_sb[:],
                                start=(st == 0), stop=(st == n_st - 1),
                            )
                        # normalize: rows 0..63 / row 64
                        rc_sb = nrm.tile([65, TC], dt.float32, name="rc_sb")
                        nc.vector.reciprocal(rc_sb[64:65, :], ps_o[64:65, :])
                        rc0_sb = nrm.tile([1, TC], dt.float32, name="rc0_sb")
                        nc.sync.dma_start(out=rc0_sb[:], in_=rc_sb[64:65, :])
                        rb_sb = nrm.tile([64, TC], dt.float32, name="rb_sb")
                        nc.gpsimd.partition_broadcast(rb_sb[:], rc0_sb[:])
                        nc.vector.tensor_tensor(
                            aoT_sb[jh:jh + 64, tsl], ps_o[0:64, :], rb_sb[:],
                            mybir.AluOpType.mult,
                        )
                # output projection for this batch's rows
                for mt in range(b * (T // P), (b + 1) * (T // P)):
                    for cc in range(C // TC):
                        ps_p = ps.tile([P, TC], dt.float32, name="ps_p", tag="ps")
                        nc.tensor.matmul(
                            ps_p[:],
                            aoT_sb[:, mt * P:(mt + 1) * P],
                            wo_sb[:, cc * TC:(cc + 1) * TC],
                            start=True, stop=True,
                        )
                        o_sb = outp.tile([P, TC], dt.float32, name="o_sb")
                        nc.vector.tensor_copy(o_sb[:], ps_p[:])
                        nc.sync.dma_start(
                            out=out.ap()[mt * P:(mt + 1) * P, cc * TC:(cc + 1) * TC],
                            in_=o_sb[:],
                        )

            for _ in range(repeat):
                emit_compute()

    nc.compile()
    return nc


def _host_inputs(src, mask, Wq, bq, Wk, bk, Wv, bv, Wo, bo):
    f32 = np.float32
    src = np.asarray(src, f32)
    srct = np.ascontiguousarray(src.reshape(M, C).T)

    # causal strip: m0[s, u] = 0 (keep) iff u >= s + 384, else MASK_NEG
    u = np.arange(1024)[None, :]
    s = np.arange(P)[:, None]
    m0 = np.where(u >= s + 384, 0.0, MASK_NEG).astype(f32)
    ident = np.eye(P, dtype=f32)

    in_maps = []
    for c in range(8):
        sl = slice(c * P, (c + 1) * P)
        wqkv = np.concatenate(
            [np.asarray(Wq, f32)[:, sl], np.asarray(Wk, f32)[:, sl],
             np.asarray(Wv, f32)[:, sl]], axis=1,
        )
        bias = np.stack(
            [np.asarray(bq, f32)[sl] * 0.125, np.asarray(bk, f32)[sl],
             np.asarray(bv, f32)[sl]], axis=1,
        ).astype(f32)
        wo_c = np.ascontiguousarray(np.asarray(Wo, f32)[sl, :])
        in_maps.append({
            "srct": srct, "wqkv": np.ascontiguousarray(wqkv), "wo": wo_c,
            "bias": np.ascontiguousarray(bias), "m0": m0, "ident": ident,
            "ones": np.ones((P, B * NST), f32),
        })
    return in_maps


def kernel(src, mask, Wq, bq, Wk, bk, Wv, bv, Wo, bo):
    from concourse.bass_utils import run_bass_kernel_spmd

    if "nc" not in _CACHE:
        _CACHE["nc"] = _build_program()
    nc = _CACHE["nc"]

    in_maps = _host_inputs(src, mask, Wq, bq, Wk, bk, Wv, bv, Wo, bo)
    res = run_bass_kernel_spmd(nc, in_maps, list(range(8)))

    acc = np.zeros((M, C), np.float64)
    for c in range(8):
        acc += res.results[c]["out"]
    acc += np.asarray(bo, np.float64)[None, :]
    return acc.astype(np.float32).reshape(B, T, C)
